# revision 21
# baseline (speedup 1.0000x reference)
"""Trainium2 Bass kernel for nn_L2GESRModule.

Reference computation:
    Fh_conv = Fh @ Wh + bh            (dead: only used via ones_like)
    ESF     = ones_like(Fh_conv)      -> gather indices are a fixed shift
    Y       = Fl @ Wl + bl
    out[b,i,j,:] = Y[b, min(i+1,H-1), min(j+1,W-1), :]

One 1x1-conv GEMM on Fl plus a static (+1,+1) clamped shift, data-parallel
over batch (1 image per core). Fh/Wh/bh are never loaded.

Transposed fp16 pipeline (rel-err gate is 2e-2; fp16 in/out costs ~4e-4):
  - Host casts Fl to fp16 and pre-transposes each image to X^T [CIN, P].
    Device computes Y^T = (X @ Wl)^T W-stationary: for cin-half kh /
    cout-half ch: psum[ch] += Wl[kh,ch]^T @ X^T[kh]. No on-chip
    transposes; X^T streams as the moving operand (N=512).
  - Flat-pixel shift out[O] = Y[O+129] is folded into the PSUM->SBUF evac
    AP offset. col-127 pixels (O%128==127) need Y[O+128] = the value at
    col O-1: a strided copy duplicates col O-1 -> O before each store.
    Output row 127 = row 126 exactly: host duplicates it (not stored).
    Bias (zeros here) is added on the host during un-transpose.
  - PSUM tiles are [128, ch=2, g=2, 512] = 4 banks; one evac instruction
    covers all 4 banks (1024 pixels x both cout halves), amortizing the
    ~200ns per-instruction engine overhead. Evacs alternate ACT/DVE.
  - 10 PE warmup matmuls on scratch data run during the DMA preamble so
    the HAM clock-gate reaches 8/8 (2.4 GHz) before real matmuls start.
  - Both HWDGE rings carry half of ALL traffic (one ring alone tops out
    ~341 GB/s; two concurrently sustain ~480+): kh0 loads + ch0 stores on
    the SP ring, W + kh1 loads + ch1 stores on the ACT ring. Load chunks
    are [1024, 3072, 4096, 4096, 4096] pixels so compute starts early.
"""

import numpy as np

import concourse.bacc as bacc
import concourse.mybir as mybir
from concourse import bass_utils, tile

B, H, W, CIN, COUT = 8, 128, 128, 256, 256
N_CORES = 8
P = H * W          # 16384 pixels per image
G = 512            # pixels per PSUM bank (fp32)
# store-chunk boundaries: small at the tail (short post-evac drain). Row 127
# ([16256,16384)) is host-duplicated, never stored.
STORE_B = [0, 2048, 4096, 8192, 12288, 14336, 16256]
CHUNKS = [4096, 4096, 4096, 4096]
WARMUP_MM = 6
f16 = mybir.dt.float16
f32 = mybir.dt.float32
f8 = mybir.dt.float8e3  # e3m4: 4 mantissa bits, rel-err ~1.3e-2 end to end


def build_nc():
    n_groups = P // G          # 32
    n_store = len(STORE_B) - 1
    # store k is safe once evacs cover cols through STORE_B[k+1]-2 (col-127
    # cells come from the fixup): evac g covers dst cols [512g-129, 512g+383)
    store_gate = [
        -(-(STORE_B[k + 1] - 384) // 512) for k in range(n_store)
    ]  # [8, 16, 24, 28, 31]
    starts = np.cumsum([0] + CHUNKS).tolist()

    nc = bacc.Bacc("TRN2", target_bir_lowering=False, debug=False)
    XT = nc.dram_tensor("XT", [2, 128, P], f8, kind="ExternalInput").ap()
    WT = nc.dram_tensor("WT", [2, 128, COUT], f16, kind="ExternalInput").ap()
    OT = nc.dram_tensor("outT", [2, 128, P], f8, kind="ExternalOutput").ap()

    with tile.TileContext(nc) as tc:
        with (
            tc.tile_pool(name="consts", bufs=1) as consts,
            tc.tile_pool(name="xt", bufs=5) as xt_pool,
            tc.tile_pool(name="ps", bufs=4, space="PSUM") as ps_pool,
        ):
            # PE warmup: keep the PE busy during the DMA preamble so the HAM
            # clock-gate is at 8/8 when real matmuls arrive. Data is garbage.
            scratch = consts.tile([128, G], f16)
            nc.vector.memset(scratch, 0.25)
            ps_warm = ps_pool.tile([128, 2, G], f32, tag="ps")
            for _ in range(WARMUP_MM):
                nc.tensor.matmul(
                    ps_warm[:, 0], scratch[:, 0:128], scratch, start=True, stop=True
                )

            w_sb = consts.tile([128, 2, COUT], f16)
            nc.scalar.dma_start(w_sb, WT.rearrange("kh p n -> p kh n"))
            out_sb = consts.tile([128, 2, P], f8)

            xt_tiles = {}

            def issue_load(c):
                t = xt_pool.tile([128, 2, max(CHUNKS)], f8, tag="xt")
                cs = CHUNKS[c]
                lo = 128 if c == 0 else 0  # Y pixels [0,129) are never used
                nc.sync.dma_start(t[:, 0, lo:cs], XT[0, :, starts[c] + lo : starts[c + 1]])
                nc.scalar.dma_start(t[:, 1, lo:cs], XT[1, :, starts[c] + lo : starts[c + 1]])
                xt_tiles[c] = t

            def fixup(sc):
                # duplicate col O-1 -> O for col-127 pixels inside store chunk
                base, hi = STORE_B[sc], STORE_B[sc + 1]
                n_t = (hi - base) // 128
                end = base + 127 + (n_t - 1) * 128 + 1
                ob = out_sb.bitcast(mybir.dt.uint8)
                for ch in (0, 1):
                    d = ob[:, ch, base + 127 : end : 128]
                    s = ob[:, ch, base + 126 : end - 1 : 128]
                    if ch == 0:
                        nc.scalar.copy(d, s)
                    else:
                        nc.vector.tensor_scalar_add(d, s, 0.0)

            def store(sc):
                # stores go out via SWDGE (gpsimd): its descriptor queue and
                # semaphores are independent of the two HWDGE rings, so store
                # triggers never head-of-line block on in-flight load lanes.
                # The last two stores ride the (by then idle) HWDGE rings in
                # parallel to shorten the post-evacuation drain.
                base, hi = STORE_B[sc], STORE_B[sc + 1]
                if sc >= n_store - 2:
                    nc.sync.dma_start(OT[0, :, base:hi], out_sb[:, 0, base:hi])
                    nc.scalar.dma_start(OT[1, :, base:hi], out_sb[:, 1, base:hi])
                else:
                    nc.gpsimd.dma_start(OT[0, :, base:hi], out_sb[:, 0, base:hi])
                    nc.gpsimd.dma_start(OT[1, :, base:hi], out_sb[:, 1, base:hi])

            for c in range(len(CHUNKS)):
                issue_load(c)
            for g in range(n_groups):
                px = g * G
                c = next(i for i in range(len(CHUNKS)) if starts[i] <= px < starts[i + 1])
                xt_t = xt_tiles[c]
                l = px - starts[c]
                ps = ps_pool.tile([128, 2, G], f32, tag="ps")
                for ch in (0, 1):
                    for kh in (0, 1):
                        nc.tensor.matmul(
                            ps[:, ch],
                            w_sb[:, kh, ch * 128 : (ch + 1) * 128],
                            xt_t[:, kh, l : l + G],
                            start=(kh == 0),
                            stop=(kh == 1),
                        )
                # evacuate both cout halves in one op, -129 shift baked in
                eng = nc.scalar if g % 2 == 1 else nc.vector
                if g == 0:
                    # leading 129 columns fall off the left edge
                    _evac(nc, eng, ps[:, :, 129:G], out_sb[:, :, 0 : G - 129])
                else:
                    d0 = px - 129
                    _evac(nc, eng, ps, out_sb[:, :, d0 : d0 + G])
                while store_gate and store_gate[0] == g:
                    store_gate.pop(0)
                    sc = n_store - len(store_gate) - 1
                    fixup(sc)
                    store(sc)
            assert not store_gate

    nc.compile()
    return nc


def _evac(nc, eng, src, dst):
    if eng is nc.scalar:
        eng.copy(dst, src)
    else:
        eng.tensor_scalar_add(dst, src, 0.0)


_cache: dict = {}


def _get_nc():
    if "nc" not in _cache:
        _cache["nc"] = build_nc()
    return _cache["nc"]


def prepare_in_maps(Fl, Wl):
    import ml_dtypes

    Fl = np.asarray(Fl, dtype=np.float32)
    WT = np.ascontiguousarray(np.asarray(Wl, dtype=np.float32).astype(np.float16))
    WT = WT.reshape(2, 128, COUT)
    in_maps = []
    for b in range(B):
        # x2 pre-scale centers randn data in e3m4's normal range (max ~15.5);
        # the host divides the output by 2 during decode
        x = (Fl[b].reshape(P, CIN) * 2.0).astype(ml_dtypes.float8_e3m4)
        xt = np.ascontiguousarray(x.T)
        in_maps.append({"XT": xt.reshape(2, 128, P), "WT": WT})
    return in_maps


def assemble_output(results, bl):
    bl = np.asarray(bl, dtype=np.float32)
    outs = []
    for b in range(B):
        yt = np.asarray(results[b]["outT"]).reshape(COUT, P)
        arr = yt.T.astype(np.float32) * 0.5        # [P, COUT], undo x2 scale
        arr[P - 128 : P] = arr[P - 256 : P - 128]  # row 127 = row 126
        if np.any(bl):
            arr += bl
        outs.append(arr.reshape(H, W, COUT))
    return np.stack(outs, axis=0)


def kernel(Fh, Fl, Wh, bh, Wl, bl):
    nc = _get_nc()
    in_maps = prepare_in_maps(Fl, Wl)
    res = bass_utils.run_bass_kernel_spmd(nc, in_maps, core_ids=list(range(N_CORES)))
    return assemble_output(res.results, bl)


# revision 22
# speedup vs baseline: 1.0398x; 1.0398x over previous
"""Trainium2 Bass kernel for nn_L2GESRModule.

Reference computation:
    Fh_conv = Fh @ Wh + bh            (dead: only used via ones_like)
    ESF     = ones_like(Fh_conv)      -> gather indices are a fixed shift
    Y       = Fl @ Wl + bl
    out[b,i,j,:] = Y[b, min(i+1,H-1), min(j+1,W-1), :]

One 1x1-conv GEMM on Fl plus a static (+1,+1) clamped shift, data-parallel
over batch (1 image per core). Fh/Wh/bh are never loaded.

Transposed fp16 pipeline (rel-err gate is 2e-2; fp16 in/out costs ~4e-4):
  - Host casts Fl to fp16 and pre-transposes each image to X^T [CIN, P].
    Device computes Y^T = (X @ Wl)^T W-stationary: for cin-half kh /
    cout-half ch: psum[ch] += Wl[kh,ch]^T @ X^T[kh]. No on-chip
    transposes; X^T streams as the moving operand (N=512).
  - Flat-pixel shift out[O] = Y[O+129] is folded into the PSUM->SBUF evac
    AP offset. col-127 pixels (O%128==127) need Y[O+128] = the value at
    col O-1: a strided copy duplicates col O-1 -> O before each store.
    Output row 127 = row 126 exactly: host duplicates it (not stored).
    Bias (zeros here) is added on the host during un-transpose.
  - PSUM tiles are [128, ch=2, g=2, 512] = 4 banks; one evac instruction
    covers all 4 banks (1024 pixels x both cout halves), amortizing the
    ~200ns per-instruction engine overhead. Evacs alternate ACT/DVE.
  - 10 PE warmup matmuls on scratch data run during the DMA preamble so
    the HAM clock-gate reaches 8/8 (2.4 GHz) before real matmuls start.
  - Both HWDGE rings carry half of ALL traffic (one ring alone tops out
    ~341 GB/s; two concurrently sustain ~480+): kh0 loads + ch0 stores on
    the SP ring, W + kh1 loads + ch1 stores on the ACT ring. Load chunks
    are [1024, 3072, 4096, 4096, 4096] pixels so compute starts early.
"""

import numpy as np

import concourse.bacc as bacc
import concourse.mybir as mybir
from concourse import bass_utils, tile

B, H, W, CIN, COUT = 8, 128, 128, 256, 256
N_CORES = 8
P = H * W          # 16384 pixels per image
G = 512            # pixels per PSUM bank (fp32)
# store-chunk boundaries: small at the tail (short post-evac drain). Row 127
# ([16256,16384)) is host-duplicated, never stored.
STORE_B = [0, 2048, 4096, 8192, 12288, 14336, 16256]
CHUNKS = [4096, 4096, 4096, 4096]
WARMUP_MM = 12
f16 = mybir.dt.float16
f32 = mybir.dt.float32
f8 = mybir.dt.float8e3  # e3m4: 4 mantissa bits, rel-err ~1.3e-2 end to end


def build_nc():
    n_groups = P // G          # 32
    n_store = len(STORE_B) - 1
    # store k is safe once evacs cover cols through STORE_B[k+1]-2 (col-127
    # cells come from the fixup): evac g covers dst cols [512g-129, 512g+383)
    store_gate = [
        -(-(STORE_B[k + 1] - 384) // 512) for k in range(n_store)
    ]  # [8, 16, 24, 28, 31]
    starts = np.cumsum([0] + CHUNKS).tolist()

    nc = bacc.Bacc("TRN2", target_bir_lowering=False, debug=False)
    XT = nc.dram_tensor("XT", [2, 128, P], f8, kind="ExternalInput").ap()
    WT = nc.dram_tensor("WT", [2, 128, COUT], f16, kind="ExternalInput").ap()
    OT = nc.dram_tensor("outT", [2, 128, P], f8, kind="ExternalOutput").ap()

    with tile.TileContext(nc) as tc:
        with (
            tc.tile_pool(name="consts", bufs=1) as consts,
            tc.tile_pool(name="xt", bufs=5) as xt_pool,
            tc.tile_pool(name="ps", bufs=4, space="PSUM") as ps_pool,
        ):
            # PE warmup: keep the PE busy during the DMA preamble so the HAM
            # clock-gate is at 8/8 when real matmuls arrive. Data is garbage.
            scratch = consts.tile([128, G], f16)
            nc.vector.memset(scratch, 0.25)
            ps_warm = ps_pool.tile([128, 2, G], f32, tag="ps")
            for _ in range(WARMUP_MM):
                nc.tensor.matmul(
                    ps_warm[:, 0], scratch[:, 0:128], scratch, start=True, stop=True
                )

            w_sb = consts.tile([128, 2, COUT], f16)
            nc.gpsimd.dma_start(w_sb, WT.rearrange("kh p n -> p kh n"))
            out_sb = consts.tile([128, 2, P], f8)

            xt_tiles = {}

            def issue_load(c):
                t = xt_pool.tile([128, 2, max(CHUNKS)], f8, tag="xt")
                cs = CHUNKS[c]
                lo = 128 if c == 0 else 0  # Y pixels [0,129) are never used
                nc.sync.dma_start(t[:, 0, lo:cs], XT[0, :, starts[c] + lo : starts[c + 1]])
                nc.scalar.dma_start(t[:, 1, lo:cs], XT[1, :, starts[c] + lo : starts[c + 1]])
                xt_tiles[c] = t

            def fixup(sc):
                # duplicate col O-1 -> O for col-127 pixels inside store chunk
                base, hi = STORE_B[sc], STORE_B[sc + 1]
                n_t = (hi - base) // 128
                end = base + 127 + (n_t - 1) * 128 + 1
                ob = out_sb.bitcast(mybir.dt.uint8)
                for ch in (0, 1):
                    d = ob[:, ch, base + 127 : end : 128]
                    s = ob[:, ch, base + 126 : end - 1 : 128]
                    if ch == 0:
                        nc.scalar.copy(d, s)
                    else:
                        nc.vector.tensor_scalar_add(d, s, 0.0)

            def store(sc):
                # stores go out via SWDGE (gpsimd): its descriptor queue and
                # semaphores are independent of the two HWDGE rings, so store
                # triggers never head-of-line block on in-flight load lanes.
                # The last two stores ride the (by then idle) HWDGE rings in
                # parallel to shorten the post-evacuation drain.
                base, hi = STORE_B[sc], STORE_B[sc + 1]
                if sc >= n_store - 2:
                    nc.sync.dma_start(OT[0, :, base:hi], out_sb[:, 0, base:hi])
                    nc.scalar.dma_start(OT[1, :, base:hi], out_sb[:, 1, base:hi])
                else:
                    nc.gpsimd.dma_start(OT[0, :, base:hi], out_sb[:, 0, base:hi])
                    nc.gpsimd.dma_start(OT[1, :, base:hi], out_sb[:, 1, base:hi])

            for c in range(len(CHUNKS)):
                issue_load(c)
            for g in range(n_groups):
                px = g * G
                c = next(i for i in range(len(CHUNKS)) if starts[i] <= px < starts[i + 1])
                xt_t = xt_tiles[c]
                l = px - starts[c]
                ps = ps_pool.tile([128, 2, G], f32, tag="ps")
                for ch in (0, 1):
                    for kh in (0, 1):
                        nc.tensor.matmul(
                            ps[:, ch],
                            w_sb[:, kh, ch * 128 : (ch + 1) * 128],
                            xt_t[:, kh, l : l + G],
                            start=(kh == 0),
                            stop=(kh == 1),
                        )
                # evacuate both cout halves in one op, -129 shift baked in
                eng = nc.scalar if g % 2 == 1 else nc.vector
                if g == 0:
                    # leading 129 columns fall off the left edge
                    _evac(nc, eng, ps[:, :, 129:G], out_sb[:, :, 0 : G - 129])
                else:
                    d0 = px - 129
                    _evac(nc, eng, ps, out_sb[:, :, d0 : d0 + G])
                while store_gate and store_gate[0] == g:
                    store_gate.pop(0)
                    sc = n_store - len(store_gate) - 1
                    fixup(sc)
                    store(sc)
            assert not store_gate

    nc.compile()
    return nc


def _evac(nc, eng, src, dst):
    if eng is nc.scalar:
        eng.copy(dst, src)
    else:
        eng.tensor_scalar_add(dst, src, 0.0)


_cache: dict = {}


def _get_nc():
    if "nc" not in _cache:
        _cache["nc"] = build_nc()
    return _cache["nc"]


def prepare_in_maps(Fl, Wl):
    import ml_dtypes

    Fl = np.asarray(Fl, dtype=np.float32)
    WT = np.ascontiguousarray(np.asarray(Wl, dtype=np.float32).astype(np.float16))
    WT = WT.reshape(2, 128, COUT)
    in_maps = []
    for b in range(B):
        # x2 pre-scale centers randn data in e3m4's normal range (max ~15.5);
        # the host divides the output by 2 during decode
        x = (Fl[b].reshape(P, CIN) * 2.0).astype(ml_dtypes.float8_e3m4)
        xt = np.ascontiguousarray(x.T)
        in_maps.append({"XT": xt.reshape(2, 128, P), "WT": WT})
    return in_maps


def assemble_output(results, bl):
    bl = np.asarray(bl, dtype=np.float32)
    outs = []
    for b in range(B):
        yt = np.asarray(results[b]["outT"]).reshape(COUT, P)
        arr = yt.T.astype(np.float32) * 0.5        # [P, COUT], undo x2 scale
        arr[P - 128 : P] = arr[P - 256 : P - 128]  # row 127 = row 126
        if np.any(bl):
            arr += bl
        outs.append(arr.reshape(H, W, COUT))
    return np.stack(outs, axis=0)


def kernel(Fh, Fl, Wh, bh, Wl, bl):
    nc = _get_nc()
    in_maps = prepare_in_maps(Fl, Wl)
    res = bass_utils.run_bass_kernel_spmd(nc, in_maps, core_ids=list(range(N_CORES)))
    return assemble_output(res.results, bl)
